# revision 30
# baseline (speedup 1.0000x reference)
"""Trainium2 Bass kernel for a 2-layer GCN graph classifier.

Strategy (pure data parallelism over graphs, per sharding hint):
  - Graphs are partitioned into 8 contiguous groups (batch vector is sorted),
    nodes/edges follow.  Each core owns the edges whose *dst* falls in its
    node range (plus self-loops).
  - Aggregation is matmul-based: per 128-edge chunk, a one-hot selection
    matrix MT[e,d] = (dstl_e == d) * norm_e contracts gathered message rows
    against dst columns on the TensorEngine.  MT slabs are built in two
    batched DVE passes per block (is_equal with broadcast APs, then mult by
    broadcast norm) instead of one tensor_scalar per chunk.
  - Edge gathers use InstDMAGatherAnt (gpsimd.dma_gather): ONE instruction
    gathers a whole superblock's rows (num_idxs up to ~12k) from a DRAM
    table, amortizing the ~1 us SWDGE fixed cost that dominated the old
    per-chunk indirect DMAs.  Indices are int16; layer 2's 102400-row h2
    table therefore gathers via 4 src-range buckets with base-offset APs.
  - Layer 1 table is host-precomputed embed @ W1 (vocab-indexed, int16 ok).
  - Layer 2 runs with flipped matmul orientation (lhsT=MT) so relu output
    is node-major directly (no PE transpose); the bias enters via one extra
    rank-1 matmul per block.
  - Mean-pool uses host-precomputed 1/count, one batched pool gather, and
    one-hot pool matmuls; head matmul + scale + bias finish it.
  - Two launches:  AB = layer-1 + h2 tables (per-core output); host
    concatenates h2 slices; C = layer-2 + mean-pool + head.
  - fp16 operands, fp32 PSUM accumulation.
"""

import sys

sys.path.insert(0, "/opt/trn_rl_repo")

import numpy as np

import concourse.bacc as bacc
import concourse.mybir as mybir
import concourse.tile as tile
from concourse.library_config import mlp

P = 128
NCORES = 8
F16 = mybir.dt.float16
F32 = mybir.dt.float32
I16 = mybir.dt.int16
AF = mybir.ActivationFunctionType
OP = mybir.AluOpType

EMB = 64
HID = 128
NCLS = 16
SBN = 8          # blocks per gather superblock
BUCKET = 32768   # int16 index range per dma_gather bucket


def _ceil(a, b):
    return -(-a // b)


def _pieces(nb):
    """Split nb blocks into 4 gather pieces; the last is routed to the
    synchronous queue 0 and issued last."""
    if nb <= 4:
        return [(i, 1) for i in range(nb)]
    a = _ceil(nb, 4)
    out = []
    o = 0
    while o < nb:
        n = min(a, nb - o)
        out.append((o, n))
        o += n
    return out


def _wrap_idx(flat):
    """Wrap a flat int16 idx stream into the [128, n/16] SBUF layout the
    Q7 dma_gather kernel reads (partition j%16, col j//16, replicated in
    all eight 16-partition groups)."""
    n = flat.shape[0]
    nc = _ceil(n, 16)
    out = np.zeros((P, nc), np.int16)
    pad = np.zeros(nc * 16, np.int16)
    pad[:n] = flat
    blk = pad.reshape(nc, 16).T  # [16, nc]
    for g in range(8):
        out[16 * g:16 * (g + 1)] = blk
    return out


# ---------------------------------------------------------------- host prep


def _prep(node_ids, edge_index, batch, n_graphs, vocab):
    N = node_ids.shape[0]
    src = np.asarray(edge_index[0], np.int64)
    dst = np.asarray(edge_index[1], np.int64)
    batch = np.asarray(batch, np.int64)
    node_ids = np.asarray(node_ids, np.int64)
    Gpc = n_graphs // NCORES
    cuts = np.searchsorted(batch, np.arange(NCORES + 1) * Gpc)
    deg = (np.bincount(dst, minlength=N) + 1).astype(np.float64)
    dinv = 1.0 / np.sqrt(deg)
    L = cuts[1:] - cuts[:-1]
    NB = int(max(_ceil(int(l), P) for l in L))
    Lpad = NB * P
    TBL = NCORES * Lpad
    NBUCK = _ceil(TBL, BUCKET)
    slot_of = np.empty(N, np.int64)
    for c in range(NCORES):
        slot_of[cuts[c]:cuts[c + 1]] = c * Lpad + np.arange(cuts[c + 1] - cuts[c])

    dstcore = np.searchsorted(cuts[1:], dst, side="right")
    percore = []
    K1 = 0
    Kb = np.zeros(NBUCK, np.int64)
    GB = _ceil(Gpc, P)
    K_pool = 0
    for c in range(NCORES):
        m = dstcore == c
        es = np.concatenate([src[m], np.arange(cuts[c], cuts[c + 1])])
        ed = np.concatenate([dst[m], np.arange(cuts[c], cuts[c + 1])])
        bid = (ed - cuts[c]) >> 7           # dst block
        # layer-1 ordering: by block (includes self-loops)
        o1 = np.argsort(bid, kind="stable")
        cnt1 = np.bincount(bid, minlength=NB)
        K1 = max(K1, int(_ceil(int(cnt1.max()), P)))
        # layer-2: self-loops handled by a diagonal matmul, not gathered
        ns = len(src[m])
        es2 = es[:ns]
        ed2 = ed[:ns]
        bid2 = bid[:ns]
        buck = slot_of[es2] // BUCKET       # layer-2 gather bucket
        o2 = np.argsort(buck * NB + bid2, kind="stable")
        cnt2 = np.bincount(buck * NB + bid2, minlength=NBUCK * NB).reshape(NBUCK, NB)
        for b in range(NBUCK):
            Kb[b] = max(Kb[b], int(_ceil(int(cnt2[b].max()), P)))
        gl = batch[cuts[c]:cuts[c + 1]] - c * Gpc
        gb = gl >> 7
        gcnts = np.bincount(gb, minlength=GB)
        K_pool = max(K_pool, int(_ceil(int(gcnts.max()), P)))
        percore.append((es, ed, bid, es2, ed2, bid2, buck, o1, cnt1, o2, cnt2, gl, gb, gcnts))

    K2 = int(Kb.sum())
    J1 = NB * K1
    J2 = NB * K2
    NSB = _ceil(NB, SBN)
    Jp = GB * K_pool

    cores = []
    for c in range(NCORES):
        es, ed, bid, es2, ed2, bid2, buck, o1, cnt1, o2, cnt2, gl, gb, gcnts = percore[c]
        Lc = cuts[c + 1] - cuts[c]
        vid = node_ids[es]
        gslot = slot_of[es]

        # ---------- layer 1: chunks [block][k], slot j = (block*K1 + k)*128 + p
        e1, d1 = es[o1], ed[o1]
        v1 = vid[o1]
        start = np.zeros(NB, np.int64)
        start[1:] = np.cumsum(cnt1)[:-1]
        rank = np.arange(len(e1)) - start[bid[o1]]
        chunk = bid[o1] * K1 + (rank >> 7)
        pp = rank & 127
        idx1_flat = np.zeros(J1 * P, np.int16)
        dstl1 = np.full((P, J1), -1.0, np.float16)
        normf1 = np.zeros((P, J1), np.float16)
        idx1_flat[chunk * P + pp] = v1.astype(np.int16)
        dstl1[pp, chunk] = (d1 - cuts[c] - ((bid[o1]) << 7)).astype(np.float16)
        normf1[pp, chunk] = (dinv[e1] * dinv[d1]).astype(np.float16)
        # idx streams wrapped per gather piece (async queues 1-3)
        segs = []
        for sb in range(NSB):
            b0 = sb * SBN
            nb = min(SBN, NB - b0)
            for (po, pn) in _pieces(nb):
                segs.append(_wrap_idx(
                    idx1_flat[(b0 + po) * K1 * P:(b0 + po + pn) * K1 * P]))
        gidx1 = np.concatenate(segs, axis=1)

        # ---------- layer 2: per superblock, chunks [bucket][block][k]
        e2, d2 = es2[o2], ed2[o2]
        b2id = bid2[o2]
        bk2 = buck[o2]
        g2 = slot_of[es2][o2]
        start2 = np.zeros((NBUCK, NB), np.int64)
        flat_cnt = cnt2.ravel()
        start2.ravel()[1:] = np.cumsum(flat_cnt)[:-1]
        rank2 = np.arange(len(e2)) - start2[bk2, b2id]
        # chunk index within (bucket, block) accumulation layout:
        # global chunk = block*K2 + bucket_off[bk] + k   (block-major overall)
        boff = np.zeros(NBUCK, np.int64)
        boff[1:] = np.cumsum(Kb)[:-1]
        chunk2 = b2id * K2 + boff[bk2] + (rank2 >> 7)
        pq = rank2 & 127
        idx2_flat = np.zeros(J2 * P, np.int16)
        dstl2 = np.full((P, J2), -1.0, np.float16)
        normf2 = np.zeros((P, J2), np.float16)
        idx2_flat[chunk2 * P + pq] = (g2 - bk2 * BUCKET).astype(np.int16)
        dstl2[pq, chunk2] = (d2 - cuts[c] - (b2id << 7)).astype(np.float16)
        normf2[pq, chunk2] = (dinv[e2] * dinv[d2]).astype(np.float16)
        # per (sb, bucket) idx streams: slots of chunks {blk in sb, bucket b}
        # destination chunk range within sb slab: [bucket-major][block][k]
        streams = []
        for sb in range(NSB):
            b0 = sb * SBN
            nb = min(SBN, NB - b0)
            for b in range(NBUCK):
                seg = np.zeros(nb * Kb[b] * P, np.int16)
                for bi in range(nb):
                    blk = b0 + bi
                    src_c0 = blk * K2 + boff[b]
                    seg[bi * Kb[b] * P:(bi + 1) * Kb[b] * P] = \
                        idx2_flat[src_c0 * P:(src_c0 + Kb[b]) * P]
                streams.append(_wrap_idx(seg))
        gidx2 = np.concatenate(streams, axis=1)
        # ---------- pooling
        gstart = np.zeros(GB, np.int64)
        gstart[1:] = np.cumsum(gcnts)[:-1]
        r = np.arange(Lc) - gstart[gb]
        chunkp = gb * K_pool + (r >> 7)
        pr = r & 127
        pidx_flat = np.zeros(Jp * P, np.int16)
        brel = np.full((P, Jp), -1.0, np.float16)
        # local node slot (row into this core's x3d) in layer... node i ->
        # slot i (0..Lc)
        pidx_flat[chunkp * P + pr] = np.arange(Lc, dtype=np.int16)
        brel[pr, chunkp] = (gl - (gb << 7)).astype(np.float16)
        gidxp = _wrap_idx(pidx_flat)
        invc = np.zeros((P, GB), np.float32)
        cnts_g = np.bincount(gl, minlength=Gpc).astype(np.float32)
        invc[:, :] = (1.0 / np.maximum(cnts_g, 1.0)).reshape(GB, P).T

        dloc = np.zeros(NB * P, np.float32)
        dloc[:Lc] = dinv[cuts[c]:cuts[c + 1]] ** 2
        nself = dloc.reshape(NB, P)
        diaga = np.zeros((P, NB, P), np.float16)
        rr = np.arange(P)
        for blk in range(NB):
            diaga[rr, blk, rr] = nself[blk].astype(np.float16)
        diaga = diaga.reshape(P, NB * P)
        cores.append(dict(
            gidx1=gidx1, aux1=np.concatenate([dstl1, normf1], 1),
            gidx2=gidx2, aux2=np.concatenate([dstl2, normf2], 1),
            gidxp=gidxp, brel=brel, invc=invc, diaga=diaga))

    meta = dict(NB=NB, K1=K1, K2=K2, Kb=tuple(int(k) for k in Kb),
                NBUCK=NBUCK, GB=GB, K_pool=K_pool, Lpad=Lpad, Gpc=Gpc,
                TBL=TBL, Vpad=_ceil(vocab, P) * P,
                G1=gidx1.shape[1], G2=gidx2.shape[1], Jp=Jp)
    return cores, meta


# ------------------------------------------------------------ program builders


def _build_mtt(nc, mt_pool, iotaK_sb, aux_sb, j0, K, Jtot, tag="mtt"):
    """Transposed one-hot*norm block [P, d=128, c=K] in two DVE passes.
    All operands are f16 with unit-stride last dims -> 2x DVE mode.
    Chunk c of the result is mtt[:, :, c] (free stride K)."""
    mtt = mt_pool.tile([P, P, K], F16, tag=tag)
    nc.vector.tensor_tensor(
        out=mtt[:, :, :],
        in0=iotaK_sb[:, :, 0:K],
        in1=aux_sb[:, j0:j0 + K].unsqueeze(1).to_broadcast([P, P, K]),
        op=OP.is_equal)
    nc.vector.tensor_tensor(
        out=mtt[:, :, :],
        in0=mtt[:, :, :],
        in1=aux_sb[:, Jtot + j0:Jtot + j0 + K].unsqueeze(1)
            .to_broadcast([P, P, K]),
        op=OP.mult)
    return mtt


def build_ab(meta):
    NB, K1, Vpad, G1 = meta["NB"], meta["K1"], meta["Vpad"], meta["G1"]
    J1 = NB * K1
    nc = bacc.Bacc("TRN2", target_bir_lowering=False, debug=False,
                   num_devices=NCORES, num_swdge_queues=4)
    embW1p = nc.dram_tensor("embW1p", [Vpad, HID], F16, kind="ExternalInput")
    W2 = nc.dram_tensor("W2", [HID, HID], F16, kind="ExternalInput")
    b1 = nc.dram_tensor("b1", [HID, 1], F32, kind="ExternalInput")
    iotaK = nc.dram_tensor("iotaK", [P, P * K1], F16, kind="ExternalInput")
    gidx1 = nc.dram_tensor("gidx1", [P, G1], I16, kind="ExternalInput")
    aux1 = nc.dram_tensor("aux1", [P, 2 * J1], F16, kind="ExternalInput")
    h2 = nc.dram_tensor("h2", [NB * P, HID], F16, kind="ExternalOutput")

    from contextlib import ExitStack
    with tile.TileContext(nc) as tc, ExitStack() as ctx:
        const_p = ctx.enter_context(tc.tile_pool(name="constp", bufs=1))
        nc.gpsimd.load_library(mlp)

        W2_sb = const_p.tile([HID, HID], F16)
        nc.sync.dma_start(W2_sb[:, :], W2[:, :])
        b1_sb = const_p.tile([HID, 1], F32)
        nc.sync.dma_start(b1_sb[:, :], b1[:, :])
        iotaK_sb = const_p.tile([P, P, K1], F16)
        nc.sync.dma_start(iotaK_sb[:, :, :], iotaK[:, :].rearrange("p (d c) -> p d c", c=K1))
        gidx_sb = const_p.tile([P, G1], I16)
        nc.sync.dma_start(gidx_sb[:, :], gidx1[:, :])
        aux_sb = const_p.tile([P, 2 * J1], F16)
        nc.sync.dma_start(aux_sb[:, :], aux1[:, :])

        msg_p = ctx.enter_context(tc.tile_pool(name="msgp", bufs=4))
        mt_p = ctx.enter_context(tc.tile_pool(name="mtp", bufs=4))
        out_p = ctx.enter_context(tc.tile_pool(name="outp", bufs=2))
        agg_ps = ctx.enter_context(tc.tile_pool(name="aggps", bufs=2, space="PSUM"))
        h2_ps = ctx.enter_context(tc.tile_pool(name="h2ps", bufs=2, space="PSUM"))

        NSB = _ceil(NB, SBN)
        colbase1 = 0
        for sb in range(NSB):
            b0 = sb * SBN
            nb = min(SBN, NB - b0)
            Js = nb * K1
            j0 = b0 * K1
            msg = msg_p.tile([P, Js, P], F16, tag="msg")
            for pi, (po, pn) in enumerate(_pieces(nb)):
                nch = pn * K1
                nc.gpsimd.dma_gather(
                    out_ap=msg[:, po * K1:po * K1 + nch, :],
                    in_ap=embW1p.ap(),
                    idxs_ap=gidx_sb[:, colbase1:colbase1 + nch * 8],
                    num_idxs=nch * P, num_idxs_reg=nch * P, elem_size=HID,
                    single_packet=False,
                    queue_num=(1 + pi) if pi < 3 else 0)
                colbase1 += nch * 8
            h2grp = out_p.tile([P, nb, P], F16, tag="h2grp")
            for bi in range(nb):
                mtt = _build_mtt(nc, mt_p, iotaK_sb, aux_sb,
                                 j0 + bi * K1, K1, J1)
                agg = agg_ps.tile([P, P], F32, tag="agg")
                for k in range(K1):
                    nc.tensor.matmul(agg[:, :], lhsT=msg[:, bi * K1 + k, :],
                                     rhs=mtt[:, :, k], start=(k == 0),
                                     stop=(k == K1 - 1))
                xT = out_p.tile([P, P], F16, tag="xT")
                nc.scalar.activation(xT[:, :], agg[:, :], AF.Relu,
                                     bias=b1_sb[:, :])
                h2ps = h2_ps.tile([P, P], F32, tag="h2p")
                nc.tensor.matmul(h2ps[:, :], lhsT=xT[:, :], rhs=W2_sb[:, :],
                                 start=True, stop=True)
                nc.scalar.activation(h2grp[:, bi, :], h2ps[:, :], AF.Copy)
            nc.sync.dma_start(
                h2[b0 * P:(b0 + nb) * P, :].rearrange("(c p) h -> p c h", p=P),
                h2grp[:, :, :])
    nc.compile()
    return nc


def build_c(meta):
    NB, K2, Kb, NBUCK = meta["NB"], meta["K2"], meta["Kb"], meta["NBUCK"]
    GB, K_pool, TBL, G2, Jp = (meta["GB"], meta["K_pool"], meta["TBL"],
                               meta["G2"], meta["Jp"])
    J2 = NB * K2
    nc = bacc.Bacc("TRN2", target_bir_lowering=False, debug=False,
                   num_devices=NCORES, num_swdge_queues=4)
    h2tab = nc.dram_tensor("h2tab", [TBL, HID], F16, kind="ExternalInput")
    h2own = nc.dram_tensor("h2own", [NB * P, HID], F16, kind="ExternalInput")
    diaga = nc.dram_tensor("diaga", [P, NB * P], F16, kind="ExternalInput")
    gidx2 = nc.dram_tensor("gidx2", [P, G2], I16, kind="ExternalInput")
    aux2 = nc.dram_tensor("aux2", [P, 2 * J2], F16, kind="ExternalInput")
    b2row = nc.dram_tensor("b2row", [P, HID], F16, kind="ExternalInput")
    bsel = nc.dram_tensor("bsel", [P, P], F16, kind="ExternalInput")
    iotaK = nc.dram_tensor("iotaK", [P, P * K2], F16, kind="ExternalInput")
    iotaP = nc.dram_tensor("iotaP", [P, P * K_pool], F16, kind="ExternalInput")
    Wout = nc.dram_tensor("Wout", [HID, NCLS], F16, kind="ExternalInput")
    bout = nc.dram_tensor("bout", [1, NCLS], F32, kind="ExternalInput")
    gidxp = nc.dram_tensor("gidxp", [P, _ceil(Jp * P, 16)], I16,
                           kind="ExternalInput")
    brel = nc.dram_tensor("brel", [P, Jp], F16, kind="ExternalInput")
    invc = nc.dram_tensor("invc", [P, GB], F32, kind="ExternalInput")
    out = nc.dram_tensor("out", [GB * P, NCLS], F32, kind="ExternalOutput")

    from contextlib import ExitStack
    with tile.TileContext(nc) as tc, ExitStack() as ctx:
        const_p = ctx.enter_context(tc.tile_pool(name="constp", bufs=1))
        dram_p = ctx.enter_context(tc.tile_pool(name="dramp", bufs=1, space="DRAM"))
        nc.gpsimd.load_library(mlp)

        iotaK_sb = const_p.tile([P, P, K2], F16)
        nc.sync.dma_start(iotaK_sb[:, :, :], iotaK[:, :].rearrange("p (d c) -> p d c", c=K2))
        iotaP_sb = const_p.tile([P, P, K_pool], F16)
        nc.sync.dma_start(iotaP_sb[:, :, :], iotaP[:, :].rearrange("p (d c) -> p d c", c=K_pool))
        b2row_sb = const_p.tile([P, HID], F16)
        nc.sync.dma_start(b2row_sb[:, :], b2row[:, :])
        diag_sb = const_p.tile([P, NB, P], F16)
        nc.sync.dma_start(diag_sb[:, :, :],
                          diaga[:, :].rearrange("p (b d) -> p b d", d=P))
        bsel_sb = const_p.tile([P, P], F16)
        nc.sync.dma_start(bsel_sb[:, :], bsel[:, :])
        Wout_sb = const_p.tile([HID, NCLS], F16)
        nc.sync.dma_start(Wout_sb[:, :], Wout[:, :])
        bout_sb = const_p.tile([1, NCLS], F32)
        nc.sync.dma_start(bout_sb[:, :], bout[:, :])
        bout_bc = const_p.tile([P, NCLS], F32)
        nc.gpsimd.partition_broadcast(bout_bc[:, :], bout_sb[:, :])
        gidx_sb = const_p.tile([P, G2], I16)
        nc.sync.dma_start(gidx_sb[:, :], gidx2[:, :])
        aux_sb = const_p.tile([P, 2 * J2], F16)
        nc.sync.dma_start(aux_sb[:, :], aux2[:, :])
        gidxp_sb = const_p.tile([P, _ceil(Jp * P, 16)], I16)
        nc.sync.dma_start(gidxp_sb[:, :], gidxp[:, :])
        brel_sb = const_p.tile([P, Jp], F16)
        nc.sync.dma_start(brel_sb[:, :], brel[:, :])
        invc_sb = const_p.tile([P, GB], F32)
        nc.sync.dma_start(invc_sb[:, :], invc[:, :])

        x3d = dram_p.tile([NB * P, HID], F16)

        msg_p = ctx.enter_context(tc.tile_pool(name="msgp", bufs=3))
        mt_p = ctx.enter_context(tc.tile_pool(name="mtp", bufs=3))
        out_p = ctx.enter_context(tc.tile_pool(name="outp", bufs=3))
        agg_ps = ctx.enter_context(tc.tile_pool(name="aggps", bufs=2, space="PSUM"))

        boff = [0]
        for b in range(NBUCK):
            boff.append(boff[-1] + Kb[b])

        NSB = _ceil(NB, SBN)
        colbase = 0
        for sb in range(NSB):
            b0 = sb * SBN
            nb = min(SBN, NB - b0)
            Js = nb * K2
            j0 = b0 * K2
            # bucketed gathers: one msg tile per bucket (queue-parallel)
            msgs = []
            for b in range(NBUCK):
                nchunks = nb * Kb[b]
                if nchunks == 0:
                    msgs.append(None)
                    continue
                base = b * BUCKET
                rows = min(BUCKET, TBL - base)
                msgb = msg_p.tile([P, nchunks, P], F16, tag=f"msg{b}")
                nc.gpsimd.dma_gather(
                    out_ap=msgb[:, :, :],
                    in_ap=h2tab[base:base + rows, :],
                    idxs_ap=gidx_sb[:, colbase:colbase + nchunks * 8],
                    num_idxs=nchunks * P, num_idxs_reg=nchunks * P,
                    elem_size=HID, single_packet=False,
                    queue_num=1 + (b + sb) % 3)
                colbase += nchunks * 8
                msgs.append(msgb)
            h2o = msg_p.tile([P, nb, P], F16, tag="h2o")
            nc.sync.dma_start(
                h2o[:, :, :],
                h2own[b0 * P:(b0 + nb) * P, :].rearrange("(c p) h -> p c h", p=P))
            x3grp = out_p.tile([P, nb, P], F16, tag="x3grp")
            for bi in range(nb):
                blk = b0 + bi
                mtt = _build_mtt(nc, mt_p, iotaK_sb, aux_sb,
                                 blk * K2, K2, J2)
                agg = agg_ps.tile([P, P], F32, tag="agg")
                first = True
                for b in range(NBUCK):
                    if Kb[b] == 0:
                        continue
                    for k in range(Kb[b]):
                        nc.tensor.matmul(
                            agg[:, :], lhsT=mtt[:, :, boff[b] + k],
                            rhs=msgs[b][:, bi * Kb[b] + k, :],
                            start=first, stop=False)
                        first = False
                nc.tensor.matmul(agg[:, :], lhsT=diag_sb[:, blk, :],
                                 rhs=h2o[:, bi, :], start=False, stop=False)
                nc.tensor.matmul(agg[:, :], lhsT=bsel_sb[:, :],
                                 rhs=b2row_sb[:, :], start=False, stop=True)
                nc.scalar.activation(x3grp[:, bi, :], agg[:, :], AF.Relu)
            nc.sync.dma_start(
                x3d[b0 * P:(b0 + nb) * P, :].rearrange("(c p) h -> p c h", p=P),
                x3grp[:, :, :])

        # ---------------- pooling + head
        pool_p = ctx.enter_context(tc.tile_pool(name="poolp", bufs=2))
        pps = ctx.enter_context(tc.tile_pool(name="poolps", bufs=2, space="PSUM"))
        hps = ctx.enter_context(tc.tile_pool(name="headps", bufs=2, space="PSUM"))
        NIP = K_pool * P
        for g in range(GB):
            x3p = pool_p.tile([P, K_pool, P], F16, tag="x3p")
            nc.gpsimd.dma_gather(
                out_ap=x3p[:, :, :], in_ap=x3d[:, :],
                idxs_ap=gidxp_sb[:, g * (NIP // 16):(g + 1) * (NIP // 16)],
                num_idxs=NIP, num_idxs_reg=NIP, elem_size=HID,
                single_packet=False, queue_num=1 + g % 3)
            mp = pool_p.tile([P, P, K_pool], F16, tag="mp")
            nc.vector.tensor_tensor(
                out=mp[:, :, :],
                in0=iotaP_sb[:, :, :],
                in1=brel_sb[:, g * K_pool:(g + 1) * K_pool].unsqueeze(1)
                    .to_broadcast([P, P, K_pool]),
                op=OP.is_equal)
            poolps = pps.tile([P, P], F32, tag="poolps")
            for k in range(K_pool):
                nc.tensor.matmul(poolps[:, :], lhsT=x3p[:, k, :],
                                 rhs=mp[:, :, k], start=(k == 0),
                                 stop=(k == K_pool - 1))
            poolT = pool_p.tile([P, P], F16, tag="poolT")
            nc.scalar.activation(poolT[:, :], poolps[:, :], AF.Copy)
            headps = hps.tile([P, NCLS], F32, tag="headps")
            nc.tensor.matmul(headps[:, :], lhsT=poolT[:, :], rhs=Wout_sb[:, :],
                             start=True, stop=True)
            osb = pool_p.tile([P, NCLS], F32, tag="osb")
            nc.vector.tensor_scalar(out=osb[:, :], in0=headps[:, :],
                                    scalar1=invc_sb[:, g:g + 1], scalar2=None,
                                    op0=OP.mult)
            osb2 = pool_p.tile([P, NCLS], F32, tag="osb2")
            nc.vector.tensor_tensor(out=osb2[:, :], in0=osb[:, :],
                                    in1=bout_bc[:, :], op=OP.add)
            nc.sync.dma_start(out[g * P:(g + 1) * P, :], osb2[:, :])
    nc.compile()
    return nc


# ---------------------------------------------------------------- entry point


_CACHE = {}
LAST_TIMES = {}


def _shared_inputs(inputs, meta):
    Vpad = meta["Vpad"]
    V = inputs["embed"].shape[0]
    embW1 = np.asarray(inputs["embed"], np.float32) @ np.asarray(
        inputs["W1"], np.float32)
    embW1p = np.zeros((Vpad, HID), np.float16)
    embW1p[:V] = embW1.astype(np.float16)
    K1, K2, K_pool = meta["K1"], meta["K2"], meta["K_pool"]
    iotaK1 = np.repeat(np.arange(P, dtype=np.float16), K1)[None, :].repeat(P, 0)
    iotaK2 = np.repeat(np.arange(P, dtype=np.float16), K2)[None, :].repeat(P, 0)
    iotaKp = np.repeat(np.arange(P, dtype=np.float16), K_pool)[None, :].repeat(P, 0)
    b2row = np.zeros((P, HID), np.float16)
    b2row[0] = np.asarray(inputs["b2"], np.float32).astype(np.float16)
    bsel = np.zeros((P, P), np.float16)
    bsel[0, :] = 1.0
    ident = np.eye(P, dtype=np.float16)
    return dict(
        embW1p=embW1p,
        W2=np.asarray(inputs["W2"], np.float16),
        Wout=np.asarray(inputs["Wout"], np.float16),
        b1=np.asarray(inputs["b1"], np.float32).reshape(HID, 1),
        b2row=b2row, bsel=bsel, ident=ident,
        bout=np.asarray(inputs["bout"], np.float32).reshape(1, NCLS),
        iotaK1=iotaK1, iotaK2=iotaK2, iotaKp=iotaKp)


def kernel(node_ids, edge_index, batch, embed, W1, b1, W2, b2, Wout, bout,
           n_graphs=8192):
    from concourse import bass_utils
    inputs = dict(embed=embed, W1=W1, b1=b1, W2=W2, b2=b2, Wout=Wout, bout=bout)
    cores, meta = _prep(node_ids, edge_index, batch, n_graphs, embed.shape[0])
    sh = _shared_inputs(inputs, meta)

    key = ("ab", meta["NB"], meta["K1"], meta["Vpad"], meta["G1"])
    if key not in _CACHE:
        _CACHE[key] = build_ab(meta)
    nc_ab = _CACHE[key]
    in_ab = [dict(embW1p=sh["embW1p"], W2=sh["W2"], b1=sh["b1"],
                  iotaK=sh["iotaK1"], gidx1=c["gidx1"], aux1=c["aux1"])
             for c in cores]
    res_ab = bass_utils.run_bass_kernel_spmd(nc_ab, in_ab, list(range(NCORES)))
    LAST_TIMES["ab"] = res_ab.exec_time_ns
    h2tab = np.concatenate([res_ab.results[c]["h2"] for c in range(NCORES)], 0)
    h2tab = np.ascontiguousarray(h2tab.astype(np.float16))

    key2 = ("c", meta["NB"], meta["K2"], meta["Kb"], meta["GB"],
            meta["K_pool"], meta["G2"])
    if key2 not in _CACHE:
        _CACHE[key2] = build_c(meta)
    nc_c = _CACHE[key2]
    Lpad = meta["Lpad"]
    in_c = [dict(h2tab=h2tab, h2own=h2tab[cc * Lpad:(cc + 1) * Lpad],
                 diaga=c["diaga"],
                 gidx2=c["gidx2"], aux2=c["aux2"],
                 b2row=sh["b2row"], bsel=sh["bsel"], iotaK=sh["iotaK2"],
                 iotaP=sh["iotaKp"], Wout=sh["Wout"], bout=sh["bout"],
                 gidxp=c["gidxp"], brel=c["brel"], invc=c["invc"])
             for cc, c in enumerate(cores)]
    res_c = bass_utils.run_bass_kernel_spmd(nc_c, in_c, list(range(NCORES)))
    LAST_TIMES["c"] = res_c.exec_time_ns
    Gpc = meta["Gpc"]
    out = np.concatenate(
        [res_c.results[c]["out"][:Gpc] for c in range(NCORES)], 0)
    return out.astype(np.float32)


# revision 32
# speedup vs baseline: 1.1474x; 1.1474x over previous
"""Trainium2 Bass kernel for a 2-layer GCN graph classifier.

Strategy (pure data parallelism over graphs, per sharding hint):
  - Graphs are partitioned into 8 contiguous groups (batch vector is sorted),
    nodes/edges follow.  Each core owns the edges whose *dst* falls in its
    node range (plus self-loops).
  - Aggregation is matmul-based: per 128-edge chunk, a one-hot selection
    matrix MT[e,d] = (dstl_e == d) * norm_e contracts gathered message rows
    against dst columns on the TensorEngine.  MT slabs are built in two
    batched DVE passes per block (is_equal with broadcast APs, then mult by
    broadcast norm) instead of one tensor_scalar per chunk.
  - Edge gathers use InstDMAGatherAnt (gpsimd.dma_gather): ONE instruction
    gathers a whole superblock's rows (num_idxs up to ~12k) from a DRAM
    table, amortizing the ~1 us SWDGE fixed cost that dominated the old
    per-chunk indirect DMAs.  Indices are int16; layer 2's 102400-row h2
    table therefore gathers via 4 src-range buckets with base-offset APs.
  - Layer 1 table is host-precomputed embed @ W1 (vocab-indexed, int16 ok).
  - Layer 2 runs with flipped matmul orientation (lhsT=MT) so relu output
    is node-major directly (no PE transpose); the bias enters via one extra
    rank-1 matmul per block.
  - Mean-pool uses host-precomputed 1/count, one batched pool gather, and
    one-hot pool matmuls; head matmul + scale + bias finish it.
  - Two launches:  AB = layer-1 + h2 tables (per-core output); host
    concatenates h2 slices; C = layer-2 + mean-pool + head.
  - fp16 operands, fp32 PSUM accumulation.
"""

import sys

sys.path.insert(0, "/opt/trn_rl_repo")

import numpy as np

import concourse.bacc as bacc
import concourse.mybir as mybir
import concourse.tile as tile
from concourse.library_config import mlp

P = 128
NCORES = 8
F16 = mybir.dt.float16
F32 = mybir.dt.float32
I16 = mybir.dt.int16
AF = mybir.ActivationFunctionType
OP = mybir.AluOpType

EMB = 64
HID = 128
NCLS = 16
SBN = 8          # blocks per gather superblock
BUCKET = 32768   # int16 index range per dma_gather bucket


def _ceil(a, b):
    return -(-a // b)


def _pieces(nb):
    """Split nb blocks into 4 gather pieces; the last is routed to the
    synchronous queue 0 and issued last."""
    if nb <= 4:
        return [(i, 1) for i in range(nb)]
    a = _ceil(nb, 4)
    out = []
    o = 0
    while o < nb:
        n = min(a, nb - o)
        out.append((o, n))
        o += n
    return out


def _wrap_idx(flat):
    """Wrap a flat int16 idx stream into the [128, n/16] SBUF layout the
    Q7 dma_gather kernel reads (partition j%16, col j//16, replicated in
    all eight 16-partition groups)."""
    n = flat.shape[0]
    nc = _ceil(n, 16)
    out = np.zeros((P, nc), np.int16)
    pad = np.zeros(nc * 16, np.int16)
    pad[:n] = flat
    blk = pad.reshape(nc, 16).T  # [16, nc]
    for g in range(8):
        out[16 * g:16 * (g + 1)] = blk
    return out


# ---------------------------------------------------------------- host prep


def _prep(node_ids, edge_index, batch, n_graphs, vocab):
    N = node_ids.shape[0]
    src = np.asarray(edge_index[0], np.int64)
    dst = np.asarray(edge_index[1], np.int64)
    batch = np.asarray(batch, np.int64)
    node_ids = np.asarray(node_ids, np.int64)
    Gpc = n_graphs // NCORES
    cuts = np.searchsorted(batch, np.arange(NCORES + 1) * Gpc)
    deg = (np.bincount(dst, minlength=N) + 1).astype(np.float64)
    dinv = 1.0 / np.sqrt(deg)
    L = cuts[1:] - cuts[:-1]
    NB = int(max(_ceil(int(l), P) for l in L))
    Lpad = NB * P
    TBL = NCORES * Lpad
    NBUCK = _ceil(TBL, BUCKET)
    slot_of = np.empty(N, np.int64)
    for c in range(NCORES):
        slot_of[cuts[c]:cuts[c + 1]] = c * Lpad + np.arange(cuts[c + 1] - cuts[c])

    dstcore = np.searchsorted(cuts[1:], dst, side="right")
    percore = []
    K1 = 0
    Kb = np.zeros(NBUCK, np.int64)
    GB = _ceil(Gpc, P)
    K_pool = 0
    for c in range(NCORES):
        m = dstcore == c
        es = np.concatenate([src[m], np.arange(cuts[c], cuts[c + 1])])
        ed = np.concatenate([dst[m], np.arange(cuts[c], cuts[c + 1])])
        bid = (ed - cuts[c]) >> 7           # dst block
        # layer-1 ordering: by block (includes self-loops)
        o1 = np.argsort(bid, kind="stable")
        cnt1 = np.bincount(bid, minlength=NB)
        K1 = max(K1, int(_ceil(int(cnt1.max()), P)))
        # layer-2: self-loops handled by a diagonal matmul, not gathered
        ns = len(src[m])
        es2 = es[:ns]
        ed2 = ed[:ns]
        bid2 = bid[:ns]
        buck = slot_of[es2] // BUCKET       # layer-2 gather bucket
        o2 = np.argsort(buck * NB + bid2, kind="stable")
        cnt2 = np.bincount(buck * NB + bid2, minlength=NBUCK * NB).reshape(NBUCK, NB)
        for b in range(NBUCK):
            Kb[b] = max(Kb[b], int(_ceil(int(cnt2[b].max()), P)))
        gl = batch[cuts[c]:cuts[c + 1]] - c * Gpc
        gb = gl >> 7
        gcnts = np.bincount(gb, minlength=GB)
        K_pool = max(K_pool, int(_ceil(int(gcnts.max()), P)))
        percore.append((es, ed, bid, es2, ed2, bid2, buck, o1, cnt1, o2, cnt2, gl, gb, gcnts))

    K2 = int(Kb.sum())
    J1 = NB * K1
    J2 = NB * K2
    NSB = _ceil(NB, SBN)
    Jp = GB * K_pool

    cores = []
    for c in range(NCORES):
        es, ed, bid, es2, ed2, bid2, buck, o1, cnt1, o2, cnt2, gl, gb, gcnts = percore[c]
        Lc = cuts[c + 1] - cuts[c]
        vid = node_ids[es]
        gslot = slot_of[es]

        # ---------- layer 1: chunks [block][k], slot j = (block*K1 + k)*128 + p
        e1, d1 = es[o1], ed[o1]
        v1 = vid[o1]
        start = np.zeros(NB, np.int64)
        start[1:] = np.cumsum(cnt1)[:-1]
        rank = np.arange(len(e1)) - start[bid[o1]]
        chunk = bid[o1] * K1 + (rank >> 7)
        pp = rank & 127
        idx1_flat = np.zeros(J1 * P, np.int16)
        dstl1 = np.full((P, J1), -1.0, np.float16)
        normf1 = np.zeros((P, J1), np.float16)
        idx1_flat[chunk * P + pp] = v1.astype(np.int16)
        dstl1[pp, chunk] = (d1 - cuts[c] - ((bid[o1]) << 7)).astype(np.float16)
        normf1[pp, chunk] = (dinv[e1] * dinv[d1]).astype(np.float16)
        # idx streams wrapped per gather piece (async queues 1-3)
        segs = []
        for sb in range(NSB):
            b0 = sb * SBN
            nb = min(SBN, NB - b0)
            for (po, pn) in _pieces(nb):
                segs.append(_wrap_idx(
                    idx1_flat[(b0 + po) * K1 * P:(b0 + po + pn) * K1 * P]))
        gidx1 = np.concatenate(segs, axis=1)

        # ---------- layer 2: per superblock, chunks [bucket][block][k]
        e2, d2 = es2[o2], ed2[o2]
        b2id = bid2[o2]
        bk2 = buck[o2]
        g2 = slot_of[es2][o2]
        start2 = np.zeros((NBUCK, NB), np.int64)
        flat_cnt = cnt2.ravel()
        start2.ravel()[1:] = np.cumsum(flat_cnt)[:-1]
        rank2 = np.arange(len(e2)) - start2[bk2, b2id]
        # chunk index within (bucket, block) accumulation layout:
        # global chunk = block*K2 + bucket_off[bk] + k   (block-major overall)
        boff = np.zeros(NBUCK, np.int64)
        boff[1:] = np.cumsum(Kb)[:-1]
        chunk2 = b2id * K2 + boff[bk2] + (rank2 >> 7)
        pq = rank2 & 127
        idx2_flat = np.zeros(J2 * P, np.int16)
        dstl2 = np.full((P, J2), -1.0, np.float16)
        normf2 = np.zeros((P, J2), np.float16)
        idx2_flat[chunk2 * P + pq] = (g2 - bk2 * BUCKET).astype(np.int16)
        dstl2[pq, chunk2] = (d2 - cuts[c] - (b2id << 7)).astype(np.float16)
        normf2[pq, chunk2] = (dinv[e2] * dinv[d2]).astype(np.float16)
        # per (sb, bucket) idx streams: slots of chunks {blk in sb, bucket b}
        # destination chunk range within sb slab: [bucket-major][block][k]
        streams = []
        for sb in range(NSB):
            b0 = sb * SBN
            nb = min(SBN, NB - b0)
            for b in range(NBUCK):
                seg = np.zeros(nb * Kb[b] * P, np.int16)
                for bi in range(nb):
                    blk = b0 + bi
                    src_c0 = blk * K2 + boff[b]
                    seg[bi * Kb[b] * P:(bi + 1) * Kb[b] * P] = \
                        idx2_flat[src_c0 * P:(src_c0 + Kb[b]) * P]
                streams.append(_wrap_idx(seg))
        gidx2 = np.concatenate(streams, axis=1)
        # ---------- pooling
        gstart = np.zeros(GB, np.int64)
        gstart[1:] = np.cumsum(gcnts)[:-1]
        r = np.arange(Lc) - gstart[gb]
        chunkp = gb * K_pool + (r >> 7)
        pr = r & 127
        pidx_flat = np.zeros(Jp * P, np.int16)
        brel = np.full((P, Jp), -1.0, np.float16)
        # local node slot (row into this core's x3d) in layer... node i ->
        # slot i (0..Lc)
        pidx_flat[chunkp * P + pr] = np.arange(Lc, dtype=np.int16)
        brel[pr, chunkp] = (gl - (gb << 7)).astype(np.float16)
        gidxp = _wrap_idx(pidx_flat)
        invc = np.zeros((P, GB), np.float32)
        cnts_g = np.bincount(gl, minlength=Gpc).astype(np.float32)
        invc[:, :] = (1.0 / np.maximum(cnts_g, 1.0)).reshape(GB, P).T

        dloc = np.zeros(NB * P, np.float32)
        dloc[:Lc] = dinv[cuts[c]:cuts[c + 1]] ** 2
        nself = dloc.reshape(NB, P)
        diaga = np.zeros((P, NB, P), np.float16)
        rr = np.arange(P)
        for blk in range(NB):
            diaga[rr, blk, rr] = nself[blk].astype(np.float16)
        diaga = diaga.reshape(P, NB * P)
        cores.append(dict(
            gidx1=gidx1, aux1=np.concatenate([dstl1, normf1], 1),
            gidx2=gidx2, aux2=np.concatenate([dstl2, normf2], 1),
            gidxp=gidxp, brel=brel, invc=invc, diaga=diaga))

    meta = dict(NB=NB, K1=K1, K2=K2, Kb=tuple(int(k) for k in Kb),
                NBUCK=NBUCK, GB=GB, K_pool=K_pool, Lpad=Lpad, Gpc=Gpc,
                TBL=TBL, Vpad=_ceil(vocab, P) * P,
                G1=gidx1.shape[1], G2=gidx2.shape[1], Jp=Jp)
    return cores, meta


# ------------------------------------------------------------ program builders


def _build_mtt(nc, mt_pool, iotaK_sb, aux_sb, j0, K, Jtot, tag="mtt"):
    """Transposed one-hot*norm block [P, d=128, c=K] in two DVE passes.
    All operands are f16 with unit-stride last dims -> 2x DVE mode.
    Chunk c of the result is mtt[:, :, c] (free stride K)."""
    mtt = mt_pool.tile([P, P, K], F16, tag=tag)
    nc.vector.tensor_tensor(
        out=mtt[:, :, :],
        in0=iotaK_sb[:, :, 0:K],
        in1=aux_sb[:, j0:j0 + K].unsqueeze(1).to_broadcast([P, P, K]),
        op=OP.is_equal)
    nc.vector.tensor_tensor(
        out=mtt[:, :, :],
        in0=mtt[:, :, :],
        in1=aux_sb[:, Jtot + j0:Jtot + j0 + K].unsqueeze(1)
            .to_broadcast([P, P, K]),
        op=OP.mult)
    return mtt


def build_ab(meta):
    NB, K1, Vpad, G1 = meta["NB"], meta["K1"], meta["Vpad"], meta["G1"]
    J1 = NB * K1
    nc = bacc.Bacc("TRN2", target_bir_lowering=False, debug=False,
                   num_devices=NCORES, num_swdge_queues=4)
    embW1p = nc.dram_tensor("embW1p", [Vpad, HID], F16, kind="ExternalInput")
    W2 = nc.dram_tensor("W2", [HID, HID], F16, kind="ExternalInput")
    b1 = nc.dram_tensor("b1", [HID, 1], F32, kind="ExternalInput")
    iotaK = nc.dram_tensor("iotaK", [P, P * K1], F16, kind="ExternalInput")
    gidx1 = nc.dram_tensor("gidx1", [P, G1], I16, kind="ExternalInput")
    aux1 = nc.dram_tensor("aux1", [P, 2 * J1], F16, kind="ExternalInput")
    h2 = nc.dram_tensor("h2", [NB * P, HID], F16, kind="ExternalOutput")

    from contextlib import ExitStack
    with tile.TileContext(nc) as tc, ExitStack() as ctx:
        const_p = ctx.enter_context(tc.tile_pool(name="constp", bufs=1))
        nc.gpsimd.load_library(mlp)

        W2_sb = const_p.tile([HID, HID], F16)
        nc.sync.dma_start(W2_sb[:, :], W2[:, :])
        b1_sb = const_p.tile([HID, 1], F32)
        nc.sync.dma_start(b1_sb[:, :], b1[:, :])
        iotaK_sb = const_p.tile([P, P, K1], F16)
        nc.sync.dma_start(iotaK_sb[:, :, :], iotaK[:, :].rearrange("p (d c) -> p d c", c=K1))
        gidx_sb = const_p.tile([P, G1], I16)
        nc.sync.dma_start(gidx_sb[:, :], gidx1[:, :])
        aux_sb = const_p.tile([P, 2 * J1], F16)
        nc.sync.dma_start(aux_sb[:, :], aux1[:, :])

        msg_p = ctx.enter_context(tc.tile_pool(name="msgp", bufs=3))
        mt_p = ctx.enter_context(tc.tile_pool(name="mtp", bufs=3))
        out_p = ctx.enter_context(tc.tile_pool(name="outp", bufs=2))
        agg_ps = ctx.enter_context(tc.tile_pool(name="aggps", bufs=2, space="PSUM"))
        h2_ps = ctx.enter_context(tc.tile_pool(name="h2ps", bufs=2, space="PSUM"))

        NSB = _ceil(NB, SBN)
        colbase1 = 0
        for sb in range(NSB):
            b0 = sb * SBN
            nb = min(SBN, NB - b0)
            Js = nb * K1
            j0 = b0 * K1
            msg = msg_p.tile([P, Js, P], F16, tag="msg")
            for pi, (po, pn) in enumerate(_pieces(nb)):
                nch = pn * K1
                nc.gpsimd.dma_gather(
                    out_ap=msg[:, po * K1:po * K1 + nch, :],
                    in_ap=embW1p.ap(),
                    idxs_ap=gidx_sb[:, colbase1:colbase1 + nch * 8],
                    num_idxs=nch * P, num_idxs_reg=nch * P, elem_size=HID,
                    single_packet=False,
                    queue_num=(1 + pi) if pi < 3 else 0)
                colbase1 += nch * 8
            h2grp = out_p.tile([P, nb, P], F16, tag="h2grp")
            for bi in range(nb):
                mtt = _build_mtt(nc, mt_p, iotaK_sb, aux_sb,
                                 j0 + bi * K1, K1, J1)
                agg = agg_ps.tile([P, P], F32, tag="agg")
                for k in range(K1):
                    nc.tensor.matmul(agg[:, :], lhsT=msg[:, bi * K1 + k, :],
                                     rhs=mtt[:, :, k], start=(k == 0),
                                     stop=(k == K1 - 1))
                xT = out_p.tile([P, P], F16, tag="xT")
                nc.scalar.activation(xT[:, :], agg[:, :], AF.Relu,
                                     bias=b1_sb[:, :])
                h2ps = h2_ps.tile([P, P], F32, tag="h2p")
                nc.tensor.matmul(h2ps[:, :], lhsT=xT[:, :], rhs=W2_sb[:, :],
                                 start=True, stop=True)
                nc.scalar.activation(h2grp[:, bi, :], h2ps[:, :], AF.Copy)
            nc.sync.dma_start(
                h2[b0 * P:(b0 + nb) * P, :].rearrange("(c p) h -> p c h", p=P),
                h2grp[:, :, :])
    nc.compile()
    return nc


def build_c(meta):
    NB, K2, Kb, NBUCK = meta["NB"], meta["K2"], meta["Kb"], meta["NBUCK"]
    GB, K_pool, TBL, G2, Jp = (meta["GB"], meta["K_pool"], meta["TBL"],
                               meta["G2"], meta["Jp"])
    J2 = NB * K2
    nc = bacc.Bacc("TRN2", target_bir_lowering=False, debug=False,
                   num_devices=NCORES, num_swdge_queues=4)
    h2tab = nc.dram_tensor("h2tab", [TBL, HID], F16, kind="ExternalInput")
    h2own = nc.dram_tensor("h2own", [NB * P, HID], F16, kind="ExternalInput")
    diaga = nc.dram_tensor("diaga", [P, NB * P], F16, kind="ExternalInput")
    gidx2 = nc.dram_tensor("gidx2", [P, G2], I16, kind="ExternalInput")
    aux2 = nc.dram_tensor("aux2", [P, 2 * J2], F16, kind="ExternalInput")
    b2row = nc.dram_tensor("b2row", [P, HID], F16, kind="ExternalInput")
    bsel = nc.dram_tensor("bsel", [P, P], F16, kind="ExternalInput")
    iotaK = nc.dram_tensor("iotaK", [P, P * K2], F16, kind="ExternalInput")
    iotaP = nc.dram_tensor("iotaP", [P, P * K_pool], F16, kind="ExternalInput")
    Wout = nc.dram_tensor("Wout", [HID, NCLS], F16, kind="ExternalInput")
    bout = nc.dram_tensor("bout", [1, NCLS], F32, kind="ExternalInput")
    gidxp = nc.dram_tensor("gidxp", [P, _ceil(Jp * P, 16)], I16,
                           kind="ExternalInput")
    brel = nc.dram_tensor("brel", [P, Jp], F16, kind="ExternalInput")
    invc = nc.dram_tensor("invc", [P, GB], F32, kind="ExternalInput")
    out = nc.dram_tensor("out", [GB * P, NCLS], F32, kind="ExternalOutput")

    from contextlib import ExitStack
    with tile.TileContext(nc) as tc, ExitStack() as ctx:
        const_p = ctx.enter_context(tc.tile_pool(name="constp", bufs=1))
        dram_p = ctx.enter_context(tc.tile_pool(name="dramp", bufs=1, space="DRAM"))
        nc.gpsimd.load_library(mlp)

        iotaK_sb = const_p.tile([P, P, K2], F16)
        nc.sync.dma_start(iotaK_sb[:, :, :], iotaK[:, :].rearrange("p (d c) -> p d c", c=K2))
        iotaP_sb = const_p.tile([P, P, K_pool], F16)
        nc.sync.dma_start(iotaP_sb[:, :, :], iotaP[:, :].rearrange("p (d c) -> p d c", c=K_pool))
        b2row_sb = const_p.tile([P, HID], F16)
        nc.sync.dma_start(b2row_sb[:, :], b2row[:, :])
        diag_sb = const_p.tile([P, NB, P], F16)
        nc.sync.dma_start(diag_sb[:, :, :],
                          diaga[:, :].rearrange("p (b d) -> p b d", d=P))
        bsel_sb = const_p.tile([P, P], F16)
        nc.sync.dma_start(bsel_sb[:, :], bsel[:, :])
        Wout_sb = const_p.tile([HID, NCLS], F16)
        nc.sync.dma_start(Wout_sb[:, :], Wout[:, :])
        bout_sb = const_p.tile([1, NCLS], F32)
        nc.sync.dma_start(bout_sb[:, :], bout[:, :])
        bout_bc = const_p.tile([P, NCLS], F32)
        nc.gpsimd.partition_broadcast(bout_bc[:, :], bout_sb[:, :])
        gidx_sb = const_p.tile([P, G2], I16)
        nc.sync.dma_start(gidx_sb[:, :], gidx2[:, :])
        aux_sb = const_p.tile([P, 2 * J2], F16)
        nc.sync.dma_start(aux_sb[:, :], aux2[:, :])
        gidxp_sb = const_p.tile([P, _ceil(Jp * P, 16)], I16)
        nc.sync.dma_start(gidxp_sb[:, :], gidxp[:, :])
        brel_sb = const_p.tile([P, Jp], F16)
        nc.sync.dma_start(brel_sb[:, :], brel[:, :])
        invc_sb = const_p.tile([P, GB], F32)
        nc.sync.dma_start(invc_sb[:, :], invc[:, :])

        x3d = dram_p.tile([NB * P, HID], F16)

        msg_p = ctx.enter_context(tc.tile_pool(name="msgp", bufs=3))
        mt_p = ctx.enter_context(tc.tile_pool(name="mtp", bufs=3))
        out_p = ctx.enter_context(tc.tile_pool(name="outp", bufs=2))
        agg_ps = ctx.enter_context(tc.tile_pool(name="aggps", bufs=2, space="PSUM"))

        boff = [0]
        for b in range(NBUCK):
            boff.append(boff[-1] + Kb[b])

        NSB = _ceil(NB, SBN)
        colbase = 0
        for sb in range(NSB):
            b0 = sb * SBN
            nb = min(SBN, NB - b0)
            Js = nb * K2
            j0 = b0 * K2
            # bucketed gathers: one msg tile per bucket (queue-parallel)
            msgs = []
            for b in range(NBUCK):
                nchunks = nb * Kb[b]
                if nchunks == 0:
                    msgs.append(None)
                    continue
                base = b * BUCKET
                rows = min(BUCKET, TBL - base)
                msgb = msg_p.tile([P, nchunks, P], F16, tag=f"msg{b}")
                nc.gpsimd.dma_gather(
                    out_ap=msgb[:, :, :],
                    in_ap=h2tab[base:base + rows, :],
                    idxs_ap=gidx_sb[:, colbase:colbase + nchunks * 8],
                    num_idxs=nchunks * P, num_idxs_reg=nchunks * P,
                    elem_size=HID, single_packet=False,
                    queue_num=1 + (b + sb) % 3)
                colbase += nchunks * 8
                msgs.append(msgb)
            h2o = msg_p.tile([P, nb, P], F16, tag="h2o")
            nc.sync.dma_start(
                h2o[:, :, :],
                h2own[b0 * P:(b0 + nb) * P, :].rearrange("(c p) h -> p c h", p=P))
            x3grp = out_p.tile([P, nb, P], F16, tag="x3grp")
            for bi in range(nb):
                blk = b0 + bi
                mtt = _build_mtt(nc, mt_p, iotaK_sb, aux_sb,
                                 blk * K2, K2, J2)
                agg = agg_ps.tile([P, P], F32, tag="agg")
                first = True
                for b in range(NBUCK):
                    if Kb[b] == 0:
                        continue
                    for k in range(Kb[b]):
                        nc.tensor.matmul(
                            agg[:, :], lhsT=mtt[:, :, boff[b] + k],
                            rhs=msgs[b][:, bi * Kb[b] + k, :],
                            start=first, stop=False)
                        first = False
                nc.tensor.matmul(agg[:, :], lhsT=diag_sb[:, blk, :],
                                 rhs=h2o[:, bi, :], start=False, stop=False)
                nc.tensor.matmul(agg[:, :], lhsT=bsel_sb[:, :],
                                 rhs=b2row_sb[:, :], start=False, stop=True)
                nc.scalar.activation(x3grp[:, bi, :], agg[:, :], AF.Relu)
            nc.sync.dma_start(
                x3d[b0 * P:(b0 + nb) * P, :].rearrange("(c p) h -> p c h", p=P),
                x3grp[:, :, :])

        # ---------------- pooling + head
        pool_p = ctx.enter_context(tc.tile_pool(name="poolp", bufs=4))
        pps = ctx.enter_context(tc.tile_pool(name="poolps", bufs=4, space="PSUM"))
        hps = ctx.enter_context(tc.tile_pool(name="headps", bufs=2, space="PSUM"))
        NIP = K_pool * P
        for g in range(GB):
            x3p = pool_p.tile([P, K_pool, P], F16, tag="x3p")
            nc.gpsimd.dma_gather(
                out_ap=x3p[:, :, :], in_ap=x3d[:, :],
                idxs_ap=gidxp_sb[:, g * (NIP // 16):(g + 1) * (NIP // 16)],
                num_idxs=NIP, num_idxs_reg=NIP, elem_size=HID,
                single_packet=False, queue_num=1 + g % 3)
            mp = pool_p.tile([P, P, K_pool], F16, tag="mp")
            nc.vector.tensor_tensor(
                out=mp[:, :, :],
                in0=iotaP_sb[:, :, :],
                in1=brel_sb[:, g * K_pool:(g + 1) * K_pool].unsqueeze(1)
                    .to_broadcast([P, P, K_pool]),
                op=OP.is_equal)
            poolps = pps.tile([P, P], F32, tag="poolps")
            for k in range(K_pool):
                nc.tensor.matmul(poolps[:, :], lhsT=x3p[:, k, :],
                                 rhs=mp[:, :, k], start=(k == 0),
                                 stop=(k == K_pool - 1))
            poolT = pool_p.tile([P, P], F16, tag="poolT")
            nc.scalar.activation(poolT[:, :], poolps[:, :], AF.Copy)
            headps = hps.tile([P, NCLS], F32, tag="headps")
            nc.tensor.matmul(headps[:, :], lhsT=poolT[:, :], rhs=Wout_sb[:, :],
                             start=True, stop=True)
            osb = pool_p.tile([P, NCLS], F32, tag="osb")
            nc.vector.tensor_scalar(out=osb[:, :], in0=headps[:, :],
                                    scalar1=invc_sb[:, g:g + 1], scalar2=None,
                                    op0=OP.mult)
            osb2 = pool_p.tile([P, NCLS], F32, tag="osb2")
            nc.vector.tensor_tensor(out=osb2[:, :], in0=osb[:, :],
                                    in1=bout_bc[:, :], op=OP.add)
            nc.sync.dma_start(out[g * P:(g + 1) * P, :], osb2[:, :])
    nc.compile()
    return nc


# ---------------------------------------------------------------- entry point


_CACHE = {}
LAST_TIMES = {}


def _shared_inputs(inputs, meta):
    Vpad = meta["Vpad"]
    V = inputs["embed"].shape[0]
    embW1 = np.asarray(inputs["embed"], np.float32) @ np.asarray(
        inputs["W1"], np.float32)
    embW1p = np.zeros((Vpad, HID), np.float16)
    embW1p[:V] = embW1.astype(np.float16)
    K1, K2, K_pool = meta["K1"], meta["K2"], meta["K_pool"]
    iotaK1 = np.repeat(np.arange(P, dtype=np.float16), K1)[None, :].repeat(P, 0)
    iotaK2 = np.repeat(np.arange(P, dtype=np.float16), K2)[None, :].repeat(P, 0)
    iotaKp = np.repeat(np.arange(P, dtype=np.float16), K_pool)[None, :].repeat(P, 0)
    b2row = np.zeros((P, HID), np.float16)
    b2row[0] = np.asarray(inputs["b2"], np.float32).astype(np.float16)
    bsel = np.zeros((P, P), np.float16)
    bsel[0, :] = 1.0
    ident = np.eye(P, dtype=np.float16)
    return dict(
        embW1p=embW1p,
        W2=np.asarray(inputs["W2"], np.float16),
        Wout=np.asarray(inputs["Wout"], np.float16),
        b1=np.asarray(inputs["b1"], np.float32).reshape(HID, 1),
        b2row=b2row, bsel=bsel, ident=ident,
        bout=np.asarray(inputs["bout"], np.float32).reshape(1, NCLS),
        iotaK1=iotaK1, iotaK2=iotaK2, iotaKp=iotaKp)


def kernel(node_ids, edge_index, batch, embed, W1, b1, W2, b2, Wout, bout,
           n_graphs=8192):
    from concourse import bass_utils
    inputs = dict(embed=embed, W1=W1, b1=b1, W2=W2, b2=b2, Wout=Wout, bout=bout)
    cores, meta = _prep(node_ids, edge_index, batch, n_graphs, embed.shape[0])
    sh = _shared_inputs(inputs, meta)

    key = ("ab", meta["NB"], meta["K1"], meta["Vpad"], meta["G1"])
    if key not in _CACHE:
        _CACHE[key] = build_ab(meta)
    nc_ab = _CACHE[key]
    in_ab = [dict(embW1p=sh["embW1p"], W2=sh["W2"], b1=sh["b1"],
                  iotaK=sh["iotaK1"], gidx1=c["gidx1"], aux1=c["aux1"])
             for c in cores]
    res_ab = bass_utils.run_bass_kernel_spmd(nc_ab, in_ab, list(range(NCORES)))
    LAST_TIMES["ab"] = res_ab.exec_time_ns
    h2tab = np.concatenate([res_ab.results[c]["h2"] for c in range(NCORES)], 0)
    h2tab = np.ascontiguousarray(h2tab.astype(np.float16))

    key2 = ("c", meta["NB"], meta["K2"], meta["Kb"], meta["GB"],
            meta["K_pool"], meta["G2"])
    if key2 not in _CACHE:
        _CACHE[key2] = build_c(meta)
    nc_c = _CACHE[key2]
    Lpad = meta["Lpad"]
    in_c = [dict(h2tab=h2tab, h2own=h2tab[cc * Lpad:(cc + 1) * Lpad],
                 diaga=c["diaga"],
                 gidx2=c["gidx2"], aux2=c["aux2"],
                 b2row=sh["b2row"], bsel=sh["bsel"], iotaK=sh["iotaK2"],
                 iotaP=sh["iotaKp"], Wout=sh["Wout"], bout=sh["bout"],
                 gidxp=c["gidxp"], brel=c["brel"], invc=c["invc"])
             for cc, c in enumerate(cores)]
    res_c = bass_utils.run_bass_kernel_spmd(nc_c, in_c, list(range(NCORES)))
    LAST_TIMES["c"] = res_c.exec_time_ns
    Gpc = meta["Gpc"]
    out = np.concatenate(
        [res_c.results[c]["out"][:Gpc] for c in range(NCORES)], 0)
    return out.astype(np.float32)
